# revision 6
# baseline (speedup 1.0000x reference)
"""Trainium2 Bass kernel for nn_Attention_41540923687523 — v2.

Reference computation (per token t, H=12 heads, Dh=64):
    qkv = x @ w_qkv + b_qkv                      # [T, 2304]
    q,k,v = split(qkv reshaped [T, H, 3Dh])      # each [T, H, Dh]
    attn[t,h,g] = softmax_g( (q[t,h]·k[t,g]) * EMBED**-0.5 )
    out[t] = concat_h( sum_g attn[t,h,g] v[t,g] ) @ w_o + b_o

Sharding: pure data-parallel over batch across 8 cores (4 batch rows =
4096 tokens per core, no collectives).

v2 moves the per-token head-attention (QK^T, softmax-weighted AV) from
the vector engine onto the tensor engine via block-diagonal 8-token
groups.  Within each 512-token block, token t belongs to group
j = t % 64 at slot tau = t // 64 (tau-major grouping) — chosen so every
layout shuffle is a single 3D DMA with 128B+ contiguous runs:

  - MM1 for Q,K runs weight-stationary: psum [(head-pair,d), t]; chunks
    are evicted fp16 and moved by one DMA each into Q' = [d, (h,t)] and
    K' = [d, (g,t)] per block (d on partitions).
  - Rows 64..72 of Q'/K' hold 9 constant "mask rows": a rank-9 expansion
    of -C*(1-delta_{tau,tau'}) appended to the contraction, so the QK
    matmul itself biases all cross-token products by -C; exp then
    underflows them to exact fp16 zeros.  Diagonal contributions cancel
    exactly (-a^2 + a^2 with the same stored fp16 a).
  - QK per group j: one 73-contraction matmul
      psum[(g,tau), (h,tau')] = K'_j-slice^T x Q'_j-slice  (+mask bias)
    4 groups per psum bank; one ACT exp -> fp16 P tile which IS the AV
    stationary (block-diagonal with exact zeros off-block).
  - MM1 for V runs token-stationary; V token-major is DMA-restructured
    (2 DMAs per 128-token tile) to V' = [(g,tau), (j, d|1)] where free
    column 64 of each group slot is constant 1: the AV matmul's output
    column 64 is then the softmax denominator for free.
  - AV per group: psum[(h,tau), d|den] = P-slice^T x V'-slice.
  - DVE takes the reciprocal of the den column and multiplies it into
    the 64 value columns (psum f32 -> fp16 SBUF ao_stage), then 2 DMAs
    per tile shuffle [(h,tau), (j,d)] back to token-major ao [t, (h,d)].
  - Transpose + MM2 exactly as v1.

b_qkv/b_o are zero in this problem; the bias path is compiled only when
they are nonzero (b_o is folded on host).
"""

import numpy as np

import concourse.bass as bass
import concourse.mybir as mybir
import concourse.tile as tile
from concourse import bacc
from concourse.bass_utils import run_bass_kernel_spmd
from concourse.masks import make_identity

EMBED = 768
HEADS = 12
DH = 64
B, S = 32, 1024
N_CORES = 8
T_CORE = (B // N_CORES) * S          # 4096 tokens per core
TBLK = 512                            # tokens per block (Q'/K'/V' scope)
SUB = 128                             # tokens per tile (token-major stages)
NE = EMBED // 128                     # 6 embed chunks
QKV = 3 * EMBED
SCALE = float(EMBED) ** -0.5
G = 8                                 # tokens per attention group
GP = HEADS * G                        # 96 partitions used by group psums
NGRP = TBLK // G                      # 64 groups per block
NSEC = NGRP // 4                      # 16 sections (4 groups each) per block
MASKC = 30.0                          # cross-token logit bias (pre-exp)
NMROW = G + 1                         # 9 mask rows
KDIM = DH + NMROW                     # 73-row contraction for QK

F16 = mybir.dt.float16
F32 = mybir.dt.float32


def build_module(t_core=T_CORE, n_cores=N_CORES, add_bias_qkv=False,
                 timing_only=False):
    nc = bacc.Bacc(
        "TRN2",
        target_bir_lowering=False,
        debug=False,
        enable_asserts=False,
        num_devices=n_cores,
    )

    nblk = t_core // TBLK
    nsub = TBLK // SUB

    kind_in = "Internal" if timing_only else "ExternalInput"
    kind_out = "Internal" if timing_only else "ExternalOutput"

    xT = nc.dram_tensor("xT", [NE, 128, t_core], F16, kind=kind_in).ap()
    # weight-stationary chunks for Q,K: chunk c holds heads (2c,2c+1) of Q
    # (c<6) or K (c-6); layout [12, 128 e-rows, 6*128 cols] e-major free
    wqk = nc.dram_tensor("w_qk", [12, 128, NE * 128], F16, kind=kind_in).ap()
    wv = nc.dram_tensor("w_v", [NE, 128, EMBED], F16, kind=kind_in).ap()
    wo = nc.dram_tensor("w_o", [NE, 128, EMBED], F16, kind=kind_in).ap()
    # mask rows for Q'/K' (rank-9 expansion of -C(1-delta))
    mrq = nc.dram_tensor("mrow_q", [NMROW, HEADS * TBLK], F16, kind=kind_in).ap()
    mrk = nc.dram_tensor("mrow_k", [NMROW, HEADS * TBLK], F16, kind=kind_in).ap()
    # DRAM scratch for the partition-crossing shuffles (ping-pong per block):
    # Dv: V' image [(tau,g), (j, d|gap)] ; Da: token-major ao image [t, (h,d)]
    dv = [nc.dram_tensor(f"scr_v{i}", [GP, NGRP * (DH + 1)], F16,
                         kind="Internal").ap() for i in range(2)]
    da = [nc.dram_tensor(f"scr_a{i}", [TBLK, EMBED], F16,
                         kind="Internal").ap() for i in range(2)]
    if add_bias_qkv:
        bq = nc.dram_tensor("b_q", [DH, HEADS], F16, kind=kind_in).ap()
        bk = nc.dram_tensor("b_k", [DH, HEADS], F16, kind=kind_in).ap()
        bv = nc.dram_tensor("b_v", [1, EMBED], F16, kind=kind_in).ap()
    out = nc.dram_tensor("out", [t_core, EMBED], F16, kind=kind_out).ap()
    if timing_only:
        dumm_in = nc.dram_tensor("dummy_in", [1, 2], F32,
                                 kind="ExternalInput").ap()
        dumm_out = nc.dram_tensor("dummy_out", [1, 2], F32,
                                  kind="ExternalOutput").ap()

    with tile.TileContext(nc) as tc:
        with (
            tc.tile_pool(name="const", bufs=1) as constp,
            tc.tile_pool(name="weights", bufs=1) as wp,
            tc.tile_pool(name="qkbig", bufs=1) as qkbigp,
            tc.tile_pool(name="xin", bufs=2) as xp,
            tc.tile_pool(name="stage", bufs=3) as stp,
            tc.tile_pool(name="vprime", bufs=2) as vpp,
            tc.tile_pool(name="psec", bufs=5) as pp,
            tc.tile_pool(name="small", bufs=2) as sp,
            tc.tile_pool(name="att", bufs=2) as atp,
            tc.tile_pool(name="outp", bufs=2) as outp,
            tc.tile_pool(name="psA", bufs=2, space="PSUM") as psA,
            tc.tile_pool(name="psB", bufs=3, space="PSUM") as psB,
            tc.tile_pool(name="psD", bufs=3, space="PSUM") as psD,
        ):
            if timing_only:
                dt_ = constp.tile([1, 2], F32)
                nc.sync.dma_start(dt_[:], dumm_in[:])
                nc.sync.dma_start(dumm_out[:], dt_[:])

            xblk_tiles = {}

            def prefetch_x(blk, split=False):
                b0 = blk * TBLK
                xblk = xp.tile([128, NE * TBLK], F16, tag="xblk", name="xblk")
                if split:
                    for e in range(NE):
                        nc.sync.dma_start(
                            xblk[:, e * TBLK : (e + 1) * TBLK],
                            xT[e, :, b0 : b0 + TBLK],
                        )
                else:
                    nc.sync.dma_start(
                        xblk.rearrange("p (e t) -> p e t", e=NE),
                        xT[:, :, b0 : b0 + TBLK].rearrange("e p t -> p e t"),
                    )
                xblk_tiles[blk] = xblk

            prefetch_x(0, split=True)

            # ---- persistent weights ----
            wqk_sb = []
            for c in range(12):
                wt = wp.tile([128, NE * 128], F16, tag=f"wqk{c}")
                nc.sync.dma_start(wt[:], wqk[c])
                wqk_sb.append(wt)
            wv_sb = []
            for e in range(NE):
                wvt = wp.tile([128, EMBED], F16, tag=f"wv{e}")
                nc.sync.dma_start(wvt[:], wv[e])
                wv_sb.append(wvt)
            if add_bias_qkv:
                bq_sb = constp.tile([DH, HEADS], F16)
                nc.sync.dma_start(bq_sb[:], bq[:])
                bk_sb = constp.tile([DH, HEADS], F16)
                nc.sync.dma_start(bk_sb[:], bk[:])
                bv_sb = constp.tile([128, EMBED], F16)
                nc.sync.dma_start(bv_sb[:], bv.partition_broadcast(128))

            # ---- Q'/K' ring buffers (mask rows loaded after A(0)) ----
            qprime = [qkbigp.tile([KDIM, HEADS * TBLK], F16, tag=f"qp{i}",
                                  name="qp")
                      for i in range(3)]
            kprime = [qkbigp.tile([KDIM, HEADS * TBLK], F16, tag=f"kp{i}",
                                  name="kp")
                      for i in range(3)]

            def emit_A(blk):
                """MM1 (ws for Q,K; ts for V), evictions, Q'/K' scatter,
                V dumps to DRAM."""
                qp = qprime[blk % 3]
                kp = kprime[blk % 3]
                if blk not in xblk_tiles:
                    prefetch_x(blk)
                xblk = xblk_tiles.pop(blk)
                if blk + 1 < nblk:
                    prefetch_x(blk + 1)

                for c in range(12):
                    ps = psA.tile([128, TBLK], F32, tag="ws", name="ps")
                    for e in range(NE):
                        nc.tensor.matmul(
                            ps[:],
                            wqk_sb[c][:, e * 128 : (e + 1) * 128],
                            xblk[:, e * TBLK : (e + 1) * TBLK],
                            start=(e == 0),
                            stop=(e == NE - 1),
                        )
                    chunk = stp.tile([128, TBLK], F16, tag="chunk", name="chunk")
                    nc.vector.tensor_copy(chunk[:], ps[:])
                    # scatter into Q'/K' free layout (tau, x, j): one DMA per
                    # head parity; inner j runs are 128B contiguous both sides
                    dst = qp if c < 6 else kp
                    dst_v = dst[0:DH, :].rearrange(
                        "p (tau x j) -> p tau x j", tau=G, x=HEADS)
                    chunk_v = chunk.rearrange("(pi d) (tau j) -> pi d tau j",
                                              pi=2, tau=G)
                    h0 = 2 * (c % 6)
                    for pi in range(2):
                        nc.sync.dma_start(
                            dst_v[:, :, h0 + pi, :],
                            chunk_v[pi],
                        )

                if add_bias_qkv:
                    for prime, b_sb in ((qp, bq_sb), (kp, bk_sb)):
                        pv = prime[0:DH, :].rearrange(
                            "p (tau h j) -> p tau h j", tau=G, h=HEADS)
                        nc.vector.tensor_add(
                            pv, pv,
                            b_sb.unsqueeze(1).unsqueeze(3).broadcast_to(
                                [DH, G, HEADS, NGRP]),
                        )

                for sub in range(nsub):
                    t0 = sub * SUB
                    v_tm = stp.tile([128, EMBED], F16, tag="vtm", name="v_tm")
                    for c0, cw in ((0, 512), (512, 256)):
                        ps = psA.tile([128, TBLK], F32, tag="ws", name="ps")
                        for e in range(NE):
                            nc.tensor.matmul(
                                ps[:, :cw],
                                xblk[:, e * TBLK + t0 : e * TBLK + t0 + SUB],
                                wv_sb[e][:, c0 : c0 + cw],
                                start=(e == 0),
                                stop=(e == NE - 1),
                            )
                        nc.vector.tensor_copy(v_tm[:, c0 : c0 + cw], ps[:, :cw])
                    if add_bias_qkv:
                        nc.vector.tensor_add(v_tm[:], v_tm[:], bv_sb[:])

                    # dump v_tm halves into the DRAM V' image (tile = tau-pair
                    # 2*sub, 2*sub+1); SBUF side is a plain contiguous
                    # 64-partition window, the shuffle lives in the DRAM AP
                    v_tm_v = v_tm.rearrange("(pi j) (g d) -> pi j g d",
                                            pi=2, g=HEADS)
                    dv_v = dv[blk % 2].rearrange(
                        "(tau g) (j c) -> tau j g c", g=HEADS, c=DH + 1)
                    for pi in range(2):
                        (nc.sync if pi == 0 else nc.gpsimd).dma_start(
                            dv_v[2 * sub + pi][:, :, 0:DH],
                            v_tm_v[pi],
                        )

            def emit_B(blk):
                """Attention middle: QK, exp, AV on PE; normalize on DVE;
                dump ao_stage to DRAM token-major."""
                qp = qprime[blk % 3]
                kp = kprime[blk % 3]

                vprime = vpp.tile([GP, NGRP * (DH + 1)], F16, tag="vp",
                                  name="vprime")
                nc.gpsimd.memset(
                    vprime.rearrange("p (j c) -> p j c", c=DH + 1)[:, :, DH:],
                    1.0,
                )
                nc.sync.dma_start(
                    vprime.rearrange("p (j c) -> p j c", c=DH + 1)[:, :, 0:DH],
                    dv[blk % 2].rearrange(
                        "p (j c) -> p j c", c=DH + 1)[:, :, 0:DH],
                )

                # group-psum row order is tau-major: rows (tau, x); the
                # group-j operand slice is a single stride-64 free dim
                ao_stage = atp.tile([GP, NGRP * DH], F16, tag="aostg",
                                    name="ao_stage")
                kp_v = kp[0:KDIM].rearrange("p (i j) -> p i j", j=NGRP)
                qp_v = qp[0:KDIM].rearrange("p (i j) -> p i j", j=NGRP)

                psecs = {}

                def emit_av(s):
                    psec = psecs.pop(s)
                    psav = psB.tile([GP, 4 * (DH + 1)], F32, tag="mid",
                                    name="psav")
                    for jj in range(4):
                        j = s * 4 + jj
                        nc.tensor.matmul(
                            psav[:, jj * (DH + 1) : (jj + 1) * (DH + 1)],
                            psec[:, jj * GP : (jj + 1) * GP],
                            vprime[:, j * (DH + 1) : (j + 1) * (DH + 1)],
                            start=True, stop=True,
                        )
                    rden = sp.tile([GP, 4], F32, tag="rden", name="rden")
                    av4 = psav.rearrange("p (j c) -> p j c", c=DH + 1)
                    nc.vector.reciprocal(
                        rden.unsqueeze(2), av4[:, :, DH : DH + 1]
                    )
                    nc.vector.tensor_mul(
                        ao_stage.rearrange("p (j d) -> p j d", d=DH)[
                            :, s * 4 : (s + 1) * 4, :
                        ],
                        av4[:, :, 0:DH],
                        rden.unsqueeze(2).broadcast_to([GP, 4, DH]),
                    )

                for sec in range(NSEC):
                    psqk = psB.tile([GP, 4 * GP], F32, tag="mid", name="psqk")
                    for jj in range(4):
                        j = sec * 4 + jj
                        nc.tensor.matmul(
                            psqk[:, jj * GP : (jj + 1) * GP],
                            kp_v[:, :, j],
                            qp_v[:, :, j],
                            start=True, stop=True,
                        )  # out rows (tau,g), free (tau,h)
                    if sec >= 4:
                        emit_av(sec - 4)
                    psec = pp.tile([GP, 4 * GP], F16, tag="psec", name="psec")
                    nc.scalar.activation(
                        psec[:], psqk[:],
                        mybir.ActivationFunctionType.Exp,
                        bias=0.0, scale=1.0,
                    )
                    psecs[sec] = psec
                    if sec % 8 == 7:
                        yield
                emit_av(NSEC - 4)
                emit_av(NSEC - 3)
                emit_av(NSEC - 2)
                emit_av(NSEC - 1)

                # dump ao_stage to the DRAM token-major image, one DMA per
                # tau (12 contiguous SBUF partitions each)
                da_v = da[blk % 2].rearrange("(tau j) (h d) -> tau h j d",
                                             tau=G, h=HEADS)
                aos_v = ao_stage.rearrange("(tau h) (j d) -> tau h j d",
                                           tau=G, d=DH)
                for tau in range(G):
                    nc.gpsimd.dma_start(da_v[tau], aos_v[tau])

            def emit_C(blk):
                """Token-major reload, transpose, MM2, output DMA."""
                b0 = blk * TBLK
                osb_blk = outp.tile([128, nsub * EMBED], F16, tag="osb",
                                    name="osb_blk")
                ao_blk = atp.tile([128, nsub * EMBED], F16, tag="ao",
                                  name="ao_blk")
                nc.sync.dma_start(
                    ao_blk.rearrange("p (s f) -> p s f", s=nsub),
                    da[blk % 2].rearrange("(s p) f -> p s f", p=128),
                )
                for sub in range(nsub):
                    ao = ao_blk[:, sub * EMBED : (sub + 1) * EMBED]

                    pst = psD.tile([128, EMBED], F16, tag="tail", name="pst")
                    for j in range(NE):
                        nc.tensor.transpose(
                            pst[:, j * 128 : (j + 1) * 128],
                            ao[:, j * 128 : (j + 1) * 128],
                            identity[:],
                        )
                    aT = atp.tile([128, EMBED], F16, tag="aT", name="aT")
                    nc.vector.tensor_copy(aT[:], pst[:])

                    po0 = psD.tile([128, 384], F32, tag="tail", name="po0")
                    po1 = psD.tile([128, 384], F32, tag="tail", name="po1")
                    for j in range(NE):
                        nc.tensor.matmul(
                            po0[:], aT[:, j * 128 : (j + 1) * 128],
                            wo_sb[j][:, 0:384],
                            start=(j == 0), stop=(j == NE - 1),
                        )
                    for j in range(NE):
                        nc.tensor.matmul(
                            po1[:], aT[:, j * 128 : (j + 1) * 128],
                            wo_sb[j][:, 384:768],
                            start=(j == 0), stop=(j == NE - 1),
                        )
                    o0 = sub * EMBED
                    nc.scalar.activation(
                        osb_blk[:, o0 : o0 + 384], po0[:],
                        mybir.ActivationFunctionType.Copy, bias=0.0, scale=1.0,
                    )
                    nc.scalar.activation(
                        osb_blk[:, o0 + 384 : o0 + 768], po1[:],
                        mybir.ActivationFunctionType.Copy, bias=0.0, scale=1.0,
                    )
                    if sub % 2 == 1:
                        yield

                for hh in range(nsub):
                    t0h = b0 + hh * SUB
                    nc.sync.dma_start(
                        out[t0h : t0h + SUB, :],
                        osb_blk[:, hh * EMBED : (hh + 1) * EMBED],
                    )

            # software-pipelined emission: PE work from the next block's
            # MM1 sits ahead of the (dependency-stalling) middle/MM2 of
            # earlier blocks in the in-order instruction streams.  A(0) is
            # emitted before the non-critical preloads (mask rows, w_o,
            # identity) so the tensor engine starts as early as possible.
            emit_A(0)
            for i in range(3):
                nc.sync.dma_start(qprime[i][DH:KDIM, :], mrq[:])
                nc.sync.dma_start(kprime[i][DH:KDIM, :], mrk[:])
            wo_sb = []
            for e in range(NE):
                wot = wp.tile([128, EMBED], F16, tag=f"wo{e}", name="wot")
                nc.sync.dma_start(wot[:], wo[e])
                wo_sb.append(wot)
            identity = constp.tile([128, 128], F16)
            make_identity(nc, identity)
            def drain(gen):
                if gen is not None:
                    for _ in gen:
                        pass

            for k in range(1, nblk + 2):
                if k < nblk:
                    emit_A(k)
                bgen = emit_B(k - 1) if 1 <= k <= nblk else None
                cgen = emit_C(k - 2) if k >= 2 else None
                drain(bgen)
                drain(cgen)

    nc.compile()
    return nc


_CACHE = {}


def _get_module(t_core, n_cores, add_bias_qkv):
    key = (t_core, n_cores, add_bias_qkv)
    if key not in _CACHE:
        _CACHE[key] = build_module(t_core, n_cores, add_bias_qkv)
    return _CACHE[key]


def _mask_rows(sign_row0):
    """[NMROW, HEADS*TBLK] fp16 mask rows: rank-9 expansion of
    -C(1-delta_{tau,tau'}) over the (tau, x, j) free layout."""
    a = np.float16(np.sqrt(MASKC))
    m = np.zeros((NMROW, HEADS * TBLK), dtype=np.float16)
    m[0, :] = np.float16(sign_row0) * a
    tau_of_pos = np.arange(HEADS * TBLK) // (HEADS * NGRP)
    for i in range(G):
        m[1 + i, :] = np.where(tau_of_pos == i, a, np.float16(0.0))
    return m


def prepare_in_maps(x, w_qkv, b_qkv, w_o, b_o):
    """Host-side prep: shard over batch, transpose x, build weight chunks."""
    x = np.asarray(x)
    w_qkv = np.asarray(w_qkv)
    b_qkv = np.asarray(b_qkv)
    w_o = np.asarray(w_o)
    b_o = np.asarray(b_o)

    bias_qkv = bool(np.any(b_qkv != 0))

    w3 = w_qkv.reshape(EMBED, HEADS, 3 * DH).astype(np.float32)
    wq = w3[:, :, 0:DH] * SCALE            # [E, H, DH], scale folded
    wk = w3[:, :, DH : 2 * DH]             # [E, H, DH]
    wv = w3[:, :, 2 * DH :]                # [E, H, DH] -> cols (g,d)

    wqk_chunks = np.zeros((12, 128, NE * 128), dtype=np.float16)
    for c in range(12):
        src = wq if c < 6 else wk
        h0 = 2 * (c % 6)
        blockT = np.concatenate(
            [src[:, h0, :], src[:, h0 + 1, :]], axis=1
        )  # [E, 128] cols (pi,d)
        for e in range(NE):
            wqk_chunks[c, :, e * 128 : (e + 1) * 128] = blockT[
                e * 128 : (e + 1) * 128, :
            ].astype(np.float16)

    wv16 = np.ascontiguousarray(
        wv.reshape(EMBED, EMBED).reshape(NE, 128, EMBED).astype(np.float16)
    )
    wo16 = np.ascontiguousarray(
        w_o.astype(np.float32).reshape(NE, 128, EMBED).astype(np.float16)
    )

    mrq = _mask_rows(-1.0)
    mrk = _mask_rows(1.0)

    extra = {}
    if bias_qkv:
        b3 = b_qkv.reshape(HEADS, 3 * DH).astype(np.float32)
        extra["b_q"] = np.ascontiguousarray(
            (b3[:, 0:DH] * SCALE).T.astype(np.float16))      # [DH, H]
        extra["b_k"] = np.ascontiguousarray(
            b3[:, DH : 2 * DH].T.astype(np.float16))         # [DH, H]
        extra["b_v"] = b3[:, 2 * DH :].reshape(1, EMBED).astype(np.float16)

    b_per = B // N_CORES
    in_maps = []
    for c in range(N_CORES):
        xs = x[c * b_per : (c + 1) * b_per].reshape(T_CORE, EMBED)
        xTc = np.ascontiguousarray(xs.T.astype(np.float16)).reshape(
            NE, 128, T_CORE
        )
        m = {"xT": xTc, "w_qk": wqk_chunks, "w_v": wv16, "w_o": wo16,
             "mrow_q": mrq, "mrow_k": mrk}
        m.update(extra)
        in_maps.append(m)
    return in_maps, bias_qkv, b_o


def kernel(x, w_qkv, b_qkv, w_o, b_o):
    in_maps, bias_qkv, b_o_np = prepare_in_maps(x, w_qkv, b_qkv, w_o, b_o)
    nc = _get_module(T_CORE, N_CORES, bias_qkv)
    res = run_bass_kernel_spmd(nc, in_maps, core_ids=list(range(N_CORES)))
    out = np.concatenate([r["out"][None] for r in res.results], axis=0)
    out = out.reshape(B, S, EMBED).astype(np.float32)
    if np.any(b_o_np != 0):
        out = out + b_o_np.astype(np.float32)
    return out
